# revision 3
# baseline (speedup 1.0000x reference)
"""Trainium2 Bass kernel for nn_MultiHeadBinaryClassifier.

Math: out = heads(relu(relu(x@W0+b0)@W1+b1)) with 32 independent heads,
each Linear(64->16)+ReLU -> Linear(16->1). The head einsums collapse to:
  h      = relu(f2 @ Wh + bh)      Wh [64, 512]   (heads_w1 re-laid-out)
  logits = h @ W2blk + b2          W2blk [512, 32] block-diagonal

Sharding: pure data-parallel over the batch dim across 8 cores.

Device layout is feature-major ("transposed"): the host pre-transposes each
x shard to [128, rows] so the contraction dim (features) sits on SBUF
partitions for every matmul; weights are the stationary operand and batch
streams through as the moving free dim. Output is produced as [32, rows]
and transposed back on the host.
"""

import numpy as np

import concourse.bacc as bacc
import concourse.mybir as mybir
from concourse.tile import TileContext
from concourse.bass_utils import run_bass_kernel_spmd

AF = mybir.ActivationFunctionType
ALU = mybir.AluOpType
F32 = mybir.dt.float32
F32R = mybir.dt.float32r

B, F, H = 131072, 128, 64
NH, HD = 32, 16
HW = NH * HD  # 512, width of the fused heads layer
N_CORES = 8
RPC = B // N_CORES  # rows per core
BT = 512            # batch tile (moving free dim; one PSUM bank of f32)
NT = RPC // BT

_NC = None


def _build():
    nc = bacc.Bacc(None, target_bir_lowering=False)

    xT = nc.dram_tensor("xT", [F, RPC], F32, kind="ExternalInput")
    W0 = nc.dram_tensor("W0", [F, H], F32, kind="ExternalInput")
    W1 = nc.dram_tensor("W1", [H, H], F32, kind="ExternalInput")
    Wh = nc.dram_tensor("Wh", [H, HW], F32, kind="ExternalInput")
    W2 = nc.dram_tensor("W2", [HW, NH], F32, kind="ExternalInput")
    b0 = nc.dram_tensor("b0", [H], F32, kind="ExternalInput")
    b1 = nc.dram_tensor("b1", [H], F32, kind="ExternalInput")
    bh = nc.dram_tensor("bh", [HW], F32, kind="ExternalInput")
    b2 = nc.dram_tensor("b2", [NH], F32, kind="ExternalInput")
    outT = nc.dram_tensor("outT", [NH, RPC], F32, kind="ExternalOutput")

    with TileContext(nc) as tc:
        with (
            tc.tile_pool(name="consts", bufs=1) as cp,
            tc.tile_pool(name="xp", bufs=4) as xp,
            tc.tile_pool(name="fp", bufs=4) as fp,
            tc.tile_pool(name="hp", bufs=6) as hp,
            tc.tile_pool(name="op", bufs=3) as op,
            tc.tile_pool(name="psf", bufs=2, space="PSUM") as psf,
            tc.tile_pool(name="psh", bufs=4, space="PSUM") as psh,
            tc.tile_pool(name="pso", bufs=2, space="PSUM") as pso,
        ):
            W0s = cp.tile([F, H], F32R)
            nc.gpsimd.dma_start(W0s[:], W0[:])
            W1s = cp.tile([H, H], F32R)
            nc.gpsimd.dma_start(W1s[:], W1[:])
            Whs = cp.tile([H, HW], F32R)
            nc.gpsimd.dma_start(Whs[:], Wh[:])
            W2s = []
            for j in range(4):
                w = cp.tile([128, NH], F32R, tag=f"w2_{j}")
                nc.gpsimd.dma_start(w[:], W2[128 * j:128 * (j + 1), :])
                W2s.append(w)
            b0s = cp.tile([H, 1], F32)
            nc.sync.dma_start(b0s[:], b0[:].rearrange("(p o) -> p o", o=1))
            b1s = cp.tile([H, 1], F32)
            nc.sync.dma_start(b1s[:], b1[:].rearrange("(p o) -> p o", o=1))
            bhs = cp.tile([128, 4], F32)
            nc.sync.dma_start(bhs[:], bh[:].rearrange("(j p) -> p j", p=128))
            b2s = cp.tile([NH, 1], F32)
            nc.sync.dma_start(b2s[:], b2[:].rearrange("(p o) -> p o", o=1))

            for t in range(NT):
                sl = slice(t * BT, (t + 1) * BT)
                xt = xp.tile([F, BT], F32R, tag="x")
                nc.gpsimd.dma_start(xt[:], xT[:, sl])

                pf1 = psf.tile([H, BT], F32, tag="pf")
                nc.tensor.matmul(pf1[:], W0s[:], xt[:], start=True, stop=True)
                f1 = fp.tile([H, BT], F32R, tag="f")
                nc.scalar.activation(f1[:], pf1[:], AF.Relu, bias=b0s[:, 0:1])

                pf2 = psf.tile([H, BT], F32, tag="pf")
                nc.tensor.matmul(pf2[:], W1s[:], f1[:], start=True, stop=True)
                f2 = fp.tile([H, BT], F32R, tag="f")
                nc.scalar.activation(f2[:], pf2[:], AF.Relu, bias=b1s[:, 0:1])

                po = pso.tile([NH, BT], F32, tag="po")
                for j in range(4):
                    ph = psh.tile([128, BT], F32, tag="ph")
                    nc.tensor.matmul(
                        ph[:], Whs[:, 128 * j:128 * (j + 1)], f2[:],
                        start=True, stop=True,
                    )
                    hj = hp.tile([128, BT], F32R, tag="h")
                    # split PSUM->SBUF evacuation across ACT and DVE
                    if j % 2 == 0:
                        nc.scalar.activation(hj[:], ph[:], AF.Relu, bias=bhs[:, j:j + 1])
                    else:
                        nc.vector.tensor_scalar(
                            hj[:], ph[:], bhs[:, j:j + 1], 0.0, ALU.add, ALU.max
                        )
                    nc.tensor.matmul(
                        po[:], W2s[j][:], hj[:],
                        start=(j == 0), stop=(j == 3),
                    )

                ot = op.tile([NH, BT], F32, tag="o")
                nc.vector.tensor_scalar(ot[:], po[:], b2s[:, 0:1], None, ALU.add)
                nc.sync.dma_start(outT[:, sl], ot[:])

    nc.compile()
    return nc


def _get_nc():
    global _NC
    if _NC is None:
        _NC = _build()
    return _NC


def _prep_in_maps(x, W0, b0, W1, b1, heads_w1, heads_b1, heads_w2, heads_b2):
    f = np.float32
    Wh = np.ascontiguousarray(
        np.asarray(heads_w1, f).transpose(1, 0, 2).reshape(H, HW))
    bh = np.ascontiguousarray(np.asarray(heads_b1, f).reshape(HW))
    w2 = np.asarray(heads_w2, f).reshape(HW)  # [n*16+d]
    W2 = np.zeros((HW, NH), f)
    W2[np.arange(HW), np.arange(HW) // HD] = w2
    b2 = np.ascontiguousarray(np.asarray(heads_b2, f).reshape(NH))
    common = {
        "W0": np.ascontiguousarray(np.asarray(W0, f)),
        "W1": np.ascontiguousarray(np.asarray(W1, f)),
        "Wh": Wh,
        "W2": W2,
        "b0": np.ascontiguousarray(np.asarray(b0, f)),
        "b1": np.ascontiguousarray(np.asarray(b1, f)),
        "bh": bh,
        "b2": b2,
    }
    x = np.asarray(x, f)
    in_maps = []
    for i in range(N_CORES):
        shard = np.ascontiguousarray(x[i * RPC:(i + 1) * RPC, :].T)
        in_maps.append({"xT": shard, **common})
    return in_maps


def _assemble(results):
    out = np.empty((B, NH), np.float32)
    for i in range(N_CORES):
        out[i * RPC:(i + 1) * RPC, :] = results[i]["outT"].T
    return out


def kernel(x, W0, b0, W1, b1, heads_w1, heads_b1, heads_w2, heads_b2):
    in_maps = _prep_in_maps(x, W0, b0, W1, b1, heads_w1, heads_b1,
                            heads_w2, heads_b2)
    res = run_bass_kernel_spmd(_get_nc(), in_maps, core_ids=list(range(N_CORES)))
    return _assemble(res.results)


def benchmark(x, W0, b0, W1, b1, heads_w1, heads_b1, heads_w2, heads_b2,
              **trace_kwargs):
    """Run with NTFF tracing; returns (out, exec_time_ns, results_obj)."""
    in_maps = _prep_in_maps(x, W0, b0, W1, b1, heads_w1, heads_b1,
                            heads_w2, heads_b2)
    res = run_bass_kernel_spmd(
        _get_nc(), in_maps, core_ids=list(range(N_CORES)), trace=True,
        **trace_kwargs,
    )
    return _assemble(res.results), res.exec_time_ns, res


# revision 4
# speedup vs baseline: 8.6026x; 8.6026x over previous
"""Trainium2 Bass kernel for nn_MultiHeadBinaryClassifier.

Math: out = heads(relu(relu(x@W0+b0)@W1+b1)) with 32 independent heads,
each Linear(64->16)+ReLU -> Linear(16->1). The head einsums collapse to:
  h      = relu(f2 @ Wh + bh)      Wh [64, 512]   (heads_w1 re-laid-out)
  logits = h @ W2blk + b2          W2blk [512, 32] block-diagonal

Sharding: pure data-parallel over the batch dim across 8 cores.

Device layout is feature-major ("transposed"): the host pre-transposes each
x shard to [128, rows] so the contraction dim (features) sits on SBUF
partitions for every matmul; weights are the stationary operand and batch
streams through as the moving free dim. Output is produced as [32, rows]
and transposed back on the host.
"""

import numpy as np

import concourse.bacc as bacc
import concourse.mybir as mybir
from concourse.tile import TileContext
from concourse.bass_utils import run_bass_kernel_spmd

AF = mybir.ActivationFunctionType
ALU = mybir.AluOpType
F32 = mybir.dt.float32
F32R = mybir.dt.float32r

B, F, H = 131072, 128, 64
NH, HD = 32, 16
HW = NH * HD  # 512, width of the fused heads layer
N_CORES = 8
RPC = B // N_CORES  # rows per core
BT = 512            # batch tile (moving free dim; one PSUM bank of f32)
NT = RPC // BT

_NC = None


def _build():
    nc = bacc.Bacc(None, target_bir_lowering=False)

    xT = nc.dram_tensor("xT", [F, RPC], F32R, kind="ExternalInput")
    W0 = nc.dram_tensor("W0", [F, H], F32R, kind="ExternalInput")
    W1 = nc.dram_tensor("W1", [H, H], F32R, kind="ExternalInput")
    Wh = nc.dram_tensor("Wh", [H, HW], F32R, kind="ExternalInput")
    W2 = nc.dram_tensor("W2", [HW, NH], F32R, kind="ExternalInput")
    b0 = nc.dram_tensor("b0", [H], F32, kind="ExternalInput")
    b1 = nc.dram_tensor("b1", [H], F32, kind="ExternalInput")
    bh = nc.dram_tensor("bh", [HW], F32, kind="ExternalInput")
    b2 = nc.dram_tensor("b2", [NH], F32, kind="ExternalInput")
    outT = nc.dram_tensor("outT", [NH, RPC], F32, kind="ExternalOutput")

    with TileContext(nc) as tc:
        with (
            tc.tile_pool(name="consts", bufs=1) as cp,
            tc.tile_pool(name="xp", bufs=4) as xp,
            tc.tile_pool(name="fp", bufs=4) as fp,
            tc.tile_pool(name="hp", bufs=6) as hp,
            tc.tile_pool(name="op", bufs=3) as op,
            tc.tile_pool(name="psf", bufs=2, space="PSUM") as psf,
            tc.tile_pool(name="psh", bufs=4, space="PSUM") as psh,
            tc.tile_pool(name="pso", bufs=2, space="PSUM") as pso,
        ):
            W0s = cp.tile([F, H], F32R)
            nc.sync.dma_start(W0s[:], W0[:])
            W1s = cp.tile([H, H], F32R)
            nc.sync.dma_start(W1s[:], W1[:])
            Whs = cp.tile([H, HW], F32R)
            nc.sync.dma_start(Whs[:], Wh[:])
            W2s = []
            for j in range(4):
                w = cp.tile([128, NH], F32R, tag=f"w2_{j}")
                nc.sync.dma_start(w[:], W2[128 * j:128 * (j + 1), :])
                W2s.append(w)
            b0s = cp.tile([H, 1], F32)
            nc.sync.dma_start(b0s[:], b0[:].rearrange("(p o) -> p o", o=1))
            b1s = cp.tile([H, 1], F32)
            nc.sync.dma_start(b1s[:], b1[:].rearrange("(p o) -> p o", o=1))
            bhs = cp.tile([128, 4], F32)
            nc.sync.dma_start(bhs[:], bh[:].rearrange("(j p) -> p j", p=128))
            b2s = cp.tile([NH, 1], F32)
            nc.sync.dma_start(b2s[:], b2[:].rearrange("(p o) -> p o", o=1))

            for t in range(NT):
                sl = slice(t * BT, (t + 1) * BT)
                xt = xp.tile([F, BT], F32R, tag="x")
                nc.sync.dma_start(xt[:], xT[:, sl])

                pf1 = psf.tile([H, BT], F32, tag="pf")
                nc.tensor.matmul(pf1[:], W0s[:], xt[:], start=True, stop=True)
                f1 = fp.tile([H, BT], F32R, tag="f")
                nc.scalar.activation(f1[:], pf1[:], AF.Relu, bias=b0s[:, 0:1])

                pf2 = psf.tile([H, BT], F32, tag="pf")
                nc.tensor.matmul(pf2[:], W1s[:], f1[:], start=True, stop=True)
                f2 = fp.tile([H, BT], F32R, tag="f")
                nc.scalar.activation(f2[:], pf2[:], AF.Relu, bias=b1s[:, 0:1])

                po = pso.tile([NH, BT], F32, tag="po")
                for j in range(4):
                    ph = psh.tile([128, BT], F32, tag="ph")
                    nc.tensor.matmul(
                        ph[:], Whs[:, 128 * j:128 * (j + 1)], f2[:],
                        start=True, stop=True,
                    )
                    hj = hp.tile([128, BT], F32R, tag="h")
                    # split PSUM->SBUF evacuation across ACT and DVE
                    if j % 2 == 0:
                        nc.scalar.activation(hj[:], ph[:], AF.Relu, bias=bhs[:, j:j + 1])
                    else:
                        nc.vector.tensor_scalar(
                            hj[:], ph[:], bhs[:, j:j + 1], 0.0, ALU.add, ALU.max
                        )
                    nc.tensor.matmul(
                        po[:], W2s[j][:], hj[:],
                        start=(j == 0), stop=(j == 3),
                    )

                ot = op.tile([NH, BT], F32, tag="o")
                nc.vector.tensor_scalar(ot[:], po[:], b2s[:, 0:1], None, ALU.add)
                nc.sync.dma_start(outT[:, sl], ot[:])

    nc.compile()
    return nc


def _get_nc():
    global _NC
    if _NC is None:
        _NC = _build()
    return _NC


def _prep_in_maps(x, W0, b0, W1, b1, heads_w1, heads_b1, heads_w2, heads_b2):
    f = np.float32
    Wh = np.ascontiguousarray(
        np.asarray(heads_w1, f).transpose(1, 0, 2).reshape(H, HW))
    bh = np.ascontiguousarray(np.asarray(heads_b1, f).reshape(HW))
    w2 = np.asarray(heads_w2, f).reshape(HW)  # [n*16+d]
    W2 = np.zeros((HW, NH), f)
    W2[np.arange(HW), np.arange(HW) // HD] = w2
    b2 = np.ascontiguousarray(np.asarray(heads_b2, f).reshape(NH))
    common = {
        "W0": np.ascontiguousarray(np.asarray(W0, f)),
        "W1": np.ascontiguousarray(np.asarray(W1, f)),
        "Wh": Wh,
        "W2": W2,
        "b0": np.ascontiguousarray(np.asarray(b0, f)),
        "b1": np.ascontiguousarray(np.asarray(b1, f)),
        "bh": bh,
        "b2": b2,
    }
    x = np.asarray(x, f)
    in_maps = []
    for i in range(N_CORES):
        shard = np.ascontiguousarray(x[i * RPC:(i + 1) * RPC, :].T)
        in_maps.append({"xT": shard, **common})
    return in_maps


def _assemble(results):
    out = np.empty((B, NH), np.float32)
    for i in range(N_CORES):
        out[i * RPC:(i + 1) * RPC, :] = results[i]["outT"].T
    return out


def kernel(x, W0, b0, W1, b1, heads_w1, heads_b1, heads_w2, heads_b2):
    in_maps = _prep_in_maps(x, W0, b0, W1, b1, heads_w1, heads_b1,
                            heads_w2, heads_b2)
    res = run_bass_kernel_spmd(_get_nc(), in_maps, core_ids=list(range(N_CORES)))
    return _assemble(res.results)


def benchmark(x, W0, b0, W1, b1, heads_w1, heads_b1, heads_w2, heads_b2,
              **trace_kwargs):
    """Run with NTFF tracing; returns (out, exec_time_ns, results_obj)."""
    in_maps = _prep_in_maps(x, W0, b0, W1, b1, heads_w1, heads_b1,
                            heads_w2, heads_b2)
    res = run_bass_kernel_spmd(
        _get_nc(), in_maps, core_ids=list(range(N_CORES)), trace=True,
        **trace_kwargs,
    )
    return _assemble(res.results), res.exec_time_ns, res


# revision 6
# speedup vs baseline: 145.3513x; 16.8961x over previous
"""Trainium2 Bass kernel for nn_MultiHeadBinaryClassifier.

Math: out = heads(relu(relu(x@W0+b0)@W1+b1)) with 32 independent heads,
each Linear(64->16)+ReLU -> Linear(16->1). The head einsums collapse to:
  h      = relu(f2 @ Wh + bh)      Wh [64, 512]   (heads_w1 re-laid-out)
  logits = h @ W2blk + b2          W2blk [512, 32] block-diagonal

Sharding: pure data-parallel over the batch dim across 8 cores.

Device layout is feature-major ("transposed"): the host pre-transposes each
x shard to [128, rows] so the contraction dim (features) sits on SBUF
partitions for every matmul; weights are the stationary operand and batch
streams through as the moving free dim. Output is produced as [32, rows]
and transposed back on the host.

Two 512-wide batch tiles (A/B) are processed per step with their small
matmuls packed into disjoint PE array quadrants via SBUF/PSUM base
partitions (tile_position is auto-derived), so PSUM->SBUF evacuations run
with all 128 partitions busy and pairs of matmuls overlap in the array.
"""

import numpy as np

import concourse.bacc as bacc
import concourse.mybir as mybir
from concourse.tile import TileContext
from concourse.bass_utils import run_bass_kernel_spmd

AF = mybir.ActivationFunctionType
ALU = mybir.AluOpType
F32 = mybir.dt.float32
F32R = mybir.dt.float32r
BF16 = mybir.dt.bfloat16
MM_DT = BF16  # matmul operand dtype: bf16 allows PE quadrant packing
              # (f32r requires PSUM dst partition 0) and halves x DMA traffic


B, F, H = 131072, 128, 64
NH, HD = 32, 16
HW = NH * HD  # 512, width of the fused heads layer
N_CORES = 8
RPC = B // N_CORES  # rows per core
BT = 512            # batch tile (moving free dim; one PSUM bank of f32)

_NC = None


def _build(repeat=1):
    nc = bacc.Bacc(None, target_bir_lowering=False)

    xT = nc.dram_tensor("xT", [F, RPC], MM_DT, kind="ExternalInput")
    W0 = nc.dram_tensor("W0", [F, H], MM_DT, kind="ExternalInput")
    W1d = nc.dram_tensor("W1d", [2 * H, H], MM_DT, kind="ExternalInput")
    Whd = nc.dram_tensor("Whd", [2 * H, HW], MM_DT, kind="ExternalInput")
    W2 = nc.dram_tensor("W2", [HW, NH], MM_DT, kind="ExternalInput")
    b0d = nc.dram_tensor("b0d", [2 * H], F32, kind="ExternalInput")
    b1d = nc.dram_tensor("b1d", [2 * H], F32, kind="ExternalInput")
    bh = nc.dram_tensor("bh", [HW], F32, kind="ExternalInput")
    b2d = nc.dram_tensor("b2d", [2 * NH], F32, kind="ExternalInput")
    outT = nc.dram_tensor("outT", [NH, RPC], F32, kind="ExternalOutput")

    NT2 = RPC // (2 * BT)

    with TileContext(nc) as tc:
        with (
            tc.tile_pool(name="consts", bufs=1) as cp,
            tc.tile_pool(name="xp", bufs=3) as xp,
            tc.tile_pool(name="fp", bufs=4) as fp,
            tc.tile_pool(name="hp", bufs=6) as hp,
            tc.tile_pool(name="op", bufs=3) as op,
            tc.tile_pool(name="psf", bufs=2, space="PSUM") as psf,
            tc.tile_pool(name="psh", bufs=4, space="PSUM") as psh,
            tc.tile_pool(name="pso", bufs=2, space="PSUM") as pso,
        ):
            W0s = cp.tile([F, H], MM_DT)
            nc.sync.dma_start(W0s[:], W0[:])
            W1s = cp.tile([2 * H, H], MM_DT)
            nc.sync.dma_start(W1s[:], W1d[:])
            Whs = cp.tile([2 * H, HW], MM_DT)
            nc.sync.dma_start(Whs[:], Whd[:])
            W2s = []
            for j in range(4):
                w = cp.tile([128, NH], MM_DT, tag=f"w2_{j}")
                nc.sync.dma_start(w[:], W2[128 * j:128 * (j + 1), :])
                W2s.append(w)
            b0s = cp.tile([2 * H, 1], F32)
            nc.sync.dma_start(b0s[:], b0d[:].rearrange("(p o) -> p o", o=1))
            b1s = cp.tile([2 * H, 1], F32)
            nc.sync.dma_start(b1s[:], b1d[:].rearrange("(p o) -> p o", o=1))
            bhs = cp.tile([128, 4], F32)
            nc.sync.dma_start(bhs[:], bh[:].rearrange("(j p) -> p j", p=128))
            b2s = cp.tile([2 * NH, 1], F32)
            nc.sync.dma_start(b2s[:], b2d[:].rearrange("(p o) -> p o", o=1))

            def relu_act(dst, src, bias):
                nc.scalar.activation(dst, src, AF.Relu, bias=bias)

            def relu_dve(dst, src, bias):
                nc.vector.tensor_scalar(dst, src, bias, 0.0, ALU.add, ALU.max)

            for _ in range(repeat):
                for s in range(NT2):
                    slAB = slice(s * 2 * BT, (s + 1) * 2 * BT)
                    slA = slice(s * 2 * BT, s * 2 * BT + BT)
                    slB = slice(s * 2 * BT + BT, (s + 1) * 2 * BT)
                    xt = xp.tile([F, 2 * BT], MM_DT, tag="x")
                    nc.sync.dma_start(xt[:], xT[:, slAB])

                    # L0: two col-packed matmuls -> pf1 [128, BT]
                    pf1 = psf.tile([128, BT], F32, tag="pf")
                    nc.tensor.matmul(pf1[0:H, :], W0s[:], xt[:, 0:BT],
                                     start=True, stop=True)
                    nc.tensor.matmul(pf1[H:2 * H, :], W0s[:], xt[:, BT:2 * BT],
                                     start=True, stop=True)
                    f1 = fp.tile([128, BT], MM_DT, tag="f")
                    relu_act(f1[:], pf1[:], b0s[:, 0:1])

                    # L1: diagonal-packed (A: rows/cols 0:64, B: rows/cols 64:128)
                    pf2 = psf.tile([128, BT], F32, tag="pf")
                    nc.tensor.matmul(pf2[0:H, :], W1s[0:H, :], f1[0:H, :],
                                     start=True, stop=True)
                    nc.tensor.matmul(pf2[H:2 * H, :], W1s[H:2 * H, :],
                                     f1[H:2 * H, :], start=True, stop=True)
                    f2 = fp.tile([128, BT], MM_DT, tag="f")
                    relu_dve(f2[:], pf2[:], b1s[:, 0:1])

                    # heads: 4 chunks x (A,B) row-packed pairs; final layer
                    # accumulates into po (A -> partitions 0:32, B -> 32:64)
                    po = pso.tile([2 * NH, BT], F32, tag="po")
                    for j in range(4):
                        cj = slice(128 * j, 128 * (j + 1))
                        phA = psh.tile([128, BT], F32, tag="ph")
                        phB = psh.tile([128, BT], F32, tag="ph")
                        nc.tensor.matmul(phA[:], Whs[0:H, cj], f2[0:H, :],
                                         start=True, stop=True)
                        nc.tensor.matmul(phB[:], Whs[H:2 * H, cj], f2[H:2 * H, :],
                                         start=True, stop=True)
                        hA = hp.tile([128, BT], MM_DT, tag="h")
                        hB = hp.tile([128, BT], MM_DT, tag="h")
                        if j % 2 == 0:
                            relu_act(hA[:], phA[:], bhs[:, j:j + 1])
                            relu_dve(hB[:], phB[:], bhs[:, j:j + 1])
                        else:
                            relu_dve(hA[:], phA[:], bhs[:, j:j + 1])
                            relu_act(hB[:], phB[:], bhs[:, j:j + 1])
                        nc.tensor.matmul(po[0:NH, :], W2s[j][:], hA[:],
                                         start=(j == 0), stop=(j == 3))
                        nc.tensor.matmul(po[NH:2 * NH, :], W2s[j][:], hB[:],
                                         start=(j == 0), stop=(j == 3))

                    ot = op.tile([2 * NH, BT], F32, tag="o")
                    nc.vector.tensor_scalar(ot[:], po[:], b2s[:, 0:1], None, ALU.add)
                    nc.sync.dma_start(outT[:, slA], ot[0:NH, :])
                    nc.sync.dma_start(outT[:, slB], ot[NH:2 * NH, :])

    nc.compile()
    return nc


def _get_nc():
    global _NC
    if _NC is None:
        _NC = _build()
    return _NC


def _prep_in_maps(x, W0, b0, W1, b1, heads_w1, heads_b1, heads_w2, heads_b2):
    import ml_dtypes
    f = np.float32
    mmdt = ml_dtypes.bfloat16
    W1 = np.asarray(W1, f)
    Wh = np.asarray(heads_w1, f).transpose(1, 0, 2).reshape(H, HW)
    bh = np.ascontiguousarray(np.asarray(heads_b1, f).reshape(HW))
    w2 = np.asarray(heads_w2, f).reshape(HW)  # [n*16+d]
    W2 = np.zeros((HW, NH), f)
    W2[np.arange(HW), np.arange(HW) // HD] = w2
    b0 = np.asarray(b0, f)
    b1 = np.asarray(b1, f)
    b2 = np.asarray(heads_b2, f).reshape(NH)
    common = {
        "W0": np.ascontiguousarray(np.asarray(W0, f)).astype(mmdt),
        "W1d": np.ascontiguousarray(np.vstack([W1, W1])).astype(mmdt),
        "Whd": np.ascontiguousarray(np.vstack([Wh, Wh])).astype(mmdt),
        "W2": W2.astype(mmdt),
        "b0d": np.concatenate([b0, b0]),
        "b1d": np.concatenate([b1, b1]),
        "bh": bh,
        "b2d": np.concatenate([b2, b2]),
    }
    x = np.asarray(x, f)
    in_maps = []
    for i in range(N_CORES):
        shard = np.ascontiguousarray(x[i * RPC:(i + 1) * RPC, :].T).astype(mmdt)
        in_maps.append({"xT": shard, **common})
    return in_maps


def _assemble(results):
    out = np.empty((B, NH), np.float32)
    for i in range(N_CORES):
        out[i * RPC:(i + 1) * RPC, :] = results[i]["outT"].T
    return out


def kernel(x, W0, b0, W1, b1, heads_w1, heads_b1, heads_w2, heads_b2):
    in_maps = _prep_in_maps(x, W0, b0, W1, b1, heads_w1, heads_b1,
                            heads_w2, heads_b2)
    res = run_bass_kernel_spmd(_get_nc(), in_maps, core_ids=list(range(N_CORES)))
    return _assemble(res.results)
